# revision 23
# baseline (speedup 1.0000x reference)
"""GNN message-passing kernel (max+mean aggregation -> linear -> log_softmax)
for Trainium2, 8 NeuronCores, dst-node sharding.

Strategy:
- Shard destination nodes: core c owns global nodes [c*12500, (c+1)*12500),
  padded to 12544 = 98*128 local slots.
- Host sorts each core's nodes by in-degree and builds a SHARED degree
  template T[p] = max over cores of the p-th sorted degree, so one SPMD
  program serves all 8 cores; per-core index data pads missing slots with a
  neutral row.
- Neighbor features are gathered on-device with indirect DMA (int32 row
  indices) from xg = concat([zeros row], x + SHIFT). The shift makes the
  zero pad row neutral for max; pads add exactly 0 to sums; the shift is
  cancelled exactly by folding -SHIFT * rowsum(W) into the bias.
- Gathered slot tiles are PE-transposed to [feat, slot] layout, then DVE
  tensor_reduce (max and add) over degree-equal segments accumulates
  agg_max / agg_sum in SBUF [128 feat, 12544 nodes].
- Projection per 128-node chunk writes z in [node, cls] directly (acc slices
  as matmul lhsT), so no output transposes; z accumulates into a [128, 98, 40]
  SBUF buffer. log_softmax runs batched over groups of 14 proj chunks (one
  Exp + one Ln instruction per group, so ACT tables load ~7x, not 98x), with
  one strided DMA out per group.

Perf note: the wall is the gather - SWDGE (GpSimd Q7) descriptor generation
runs at ~8.5ns/descriptor plus ~310ns/instruction dispatch, and indirect
DMA supports only 128 rows (one per partition) per instruction, giving
~1.4us per 128 rows -> ~2.2ms for ~203k rows/core. dma_gather amortizes
dispatch (8.1ns/row at 4096 idx/instr) but its int16 indices cap the table
at 32768 rows; composing 4 windows breaks either the compile-time segment
reduce layout (padding inflation) or multiplies DVE/PE work 4x. Exec time
also varies ~20% run-to-run with the chip clock (1090 vs 1310ns per gather).
"""

import os
import sys

os.environ.setdefault("NEURON_RT_RESET_CORES", "1")
if "/opt/trn_rl_repo" not in sys.path:
    sys.path.insert(0, "/opt/trn_rl_repo")

import numpy as np

import concourse.mybir as mybir
from concourse import bacc, bass, tile
from concourse.masks import make_identity

N_NODES = 100000
D = 128
NCLS = 40
NCORES = 8
NPC = 12500
NPAD = 12544  # 98 * 128
NPROJ = NPAD // 128  # 98
CHUNK = 1536  # gather-chunk slots
IPC = CHUNK // 128  # indirect instrs per chunk
SHIFT = 12.0

last_exec_time_ns = None


def _plan(dst):
    """Global degree sort dealt 8 ways + shared template + chunk layout.

    Dealing consecutive global-rank nodes to the 8 cores puts nodes of
    (nearly) identical degree at the same template position on every core,
    so T = max-over-cores is tight (~200016 slots vs ~200901 blockwise)."""
    deg = np.bincount(dst, minlength=N_NODES).astype(np.int64)
    r = np.argsort(deg, kind="stable")  # global rank -> node id
    node_core = np.empty(N_NODES, np.int64)
    node_pos = np.empty(N_NODES, np.int64)
    node_core[r] = np.arange(N_NODES) % NCORES
    node_pos[r] = (NPAD - N_NODES // NCORES) + np.arange(N_NODES) // NCORES
    node_of = np.full((NCORES, NPAD), -1, np.int64)
    node_of[node_core, node_pos] = np.arange(N_NODES)
    degs = np.zeros((NCORES, NPAD), np.int64)
    msk = node_of >= 0
    degs[msk] = deg[node_of[msk]]
    T = degs.max(axis=0)

    # dense packing: nodes laid contiguously with no inter-chunk padding;
    # a node may straddle a chunk boundary (head fragment reduces normally,
    # tail fragment reduces into a temp then max/add-combines into acc).
    # All padding lands in the final partial chunk, whose unused columns
    # are simply never emitted.
    total_used = int(T.sum())
    ncols = (total_used + 127) // 128
    nchunks = (ncols + IPC - 1) // IPC
    node_slot_start = np.concatenate([[0], np.cumsum(T)[:-1]])
    node_end = node_slot_start + T

    pieces = []  # per chunk: (off_in_chunk, col0, nb, d, combine)
    proj_b = []  # per chunk: count of nodes finalized by its end
    for ci in range(nchunks):
        s0 = ci * CHUNK
        used_end = min((ci + 1) * CHUNK, total_used)
        pl = []
        lo = int(np.searchsorted(node_end, s0, 'right'))
        hi = int(np.searchsorted(node_slot_start, used_end, 'left'))
        i = lo
        while i < hi:
            st, en = int(node_slot_start[i]), int(node_end[i])
            cst, cen = max(st, s0), min(en, used_end)
            if cen <= cst:
                i += 1
                continue
            if st < s0 or en > used_end:
                pl.append((cst - s0, i, 1, cen - cst, st < s0))
                i += 1
            else:
                j = i
                while j < hi and int(node_end[j]) <= used_end \
                        and T[j] == T[i]:
                    j += 1
                pl.append((cst - s0, i, int(j - i), int(T[i]), False))
                i = j
        pieces.append(pl)
        proj_b.append(int(np.searchsorted(node_end, used_end, 'right')))
    return node_core, node_pos, node_of, degs, T, ncols, pieces, proj_b, \
        node_slot_start


def _core_idx(src_c, pos_c, sdeg_c, node_slot_start, total_slots):
    """int32 slot->xg-row index array for one core (0 = neutral pad row).

    pos_c: template position of each edge's dst node on this core."""
    eorder = np.argsort(pos_c, kind="stable")
    s_sorted = src_c[eorder]
    first = np.concatenate([[0], np.cumsum(sdeg_c)[:-1]])
    rank = np.arange(len(s_sorted)) - np.repeat(first, sdeg_c)
    positions = np.repeat(node_slot_start, sdeg_c) + rank
    idx = np.zeros(total_slots, np.int64)
    idx[positions] = s_sorted + 1
    return idx


def _build_program(ncols, pieces, proj_b):
    nc = bacc.Bacc()
    f32 = mybir.dt.float32
    nchunks = len(pieces)

    # projection chunk pc is ready once gather chunk ci finalizes all acc
    # cols < (pc+1)*128; emit it right after that chunk's reduces
    proj_after = [[] for _ in range(nchunks)]
    pc = 0
    for ci in range(nchunks):
        while pc < NPROJ and (pc + 1) * 128 <= proj_b[ci]:
            proj_after[ci].append(pc)
            pc += 1
    while pc < NPROJ:
        proj_after[-1].append(pc)
        pc += 1

    xg_in = nc.declare_dram_parameter("xg", [N_NODES + 1, D], f32, isOutput=False)
    idx_in = nc.declare_dram_parameter("idx", [128, ncols], mybir.dt.int32,
                                       isOutput=False)
    xT_in = nc.declare_dram_parameter("xT", [D, NPAD], f32, isOutput=False)
    invd_in = nc.declare_dram_parameter("invd", [128, NPROJ], f32, isOutput=False)
    fix_in = nc.declare_dram_parameter("fix", [128, NPROJ, NCLS], f32,
                                       isOutput=False)
    wlmaxT_in = nc.declare_dram_parameter("wlmaxT", [D, NCLS], f32, isOutput=False)
    wlmeanT_in = nc.declare_dram_parameter("wlmeanT", [D, NCLS], f32,
                                           isOutput=False)
    wrcT_in = nc.declare_dram_parameter("wrcT", [D, NCLS], f32, isOutput=False)
    o_out = nc.declare_dram_parameter("out", [NPAD, NCLS], f32, isOutput=True)

    with tile.TileContext(nc) as tc:
        with tc.tile_pool(name="persist", bufs=1) as pers:
            idx_t = pers.tile([128, ncols], mybir.dt.int32)
            invd_t = pers.tile([128, NPROJ], f32)
            fix_t = pers.tile([128, NPROJ, NCLS], f32)
            wlmaxT_t = pers.tile([D, NCLS], f32)
            wlmeanT_t = pers.tile([D, NCLS], f32)
            wrcT_t = pers.tile([D, NCLS], f32)
            ident_t = pers.tile([128, 128], f32)
            acc_max = pers.tile([128, NPAD], f32)
            acc_sum = pers.tile([128, NPAD], f32)
            zbuf = pers.tile([128, NPROJ, NCLS], f32)
            ebuf = pers.tile([128, NPROJ, NCLS], f32)
            m_t = pers.tile([128, NPROJ], f32)
            se_t = pers.tile([128, NPROJ], f32)
            ls_t = pers.tile([128, NPROJ], f32)

            # split the idx load so the first chunks' gathers start before
            # the whole 800KB index table lands
            c0 = 4 * IPC
            nc.sync.dma_start(out=idx_t[:, :c0], in_=idx_in[:, :c0])
            nc.sync.dma_start(out=idx_t[:, c0:], in_=idx_in[:, c0:])
            nc.sync.dma_start(out=invd_t[:, :], in_=invd_in[:, :])
            nc.sync.dma_start(out=fix_t[:, :, :], in_=fix_in[:, :, :])
            nc.sync.dma_start(out=wlmaxT_t[:, :], in_=wlmaxT_in[:, :])
            nc.sync.dma_start(out=wlmeanT_t[:, :], in_=wlmeanT_in[:, :])
            nc.sync.dma_start(out=wrcT_t[:, :], in_=wrcT_in[:, :])
            make_identity(nc, ident_t)
            nc.vector.memset(acc_max[:, :], 0.0)
            nc.vector.memset(acc_sum[:, :], 0.0)

            with tc.tile_pool(name="gath", bufs=4) as gpool, \
                 tc.tile_pool(name="gpsum", bufs=2, space="PSUM") as ppool, \
                 tc.tile_pool(name="proj", bufs=2) as proj, \
                 tc.tile_pool(name="ppsum", bufs=2, space="PSUM") as prps:

                def emit_proj(pc):
                    # z[node, cls] built directly: PSUM preloaded with the
                    # fix/bias, matmuls take acc slices as lhsT so no output
                    # transpose is needed; log_softmax happens once at the end
                    c0 = pc * 128
                    xT_t = proj.tile([D, 128], f32, name="xTc")
                    nc.sync.dma_start(out=xT_t[:, :], in_=xT_in[:, c0:c0 + 128])

                    ps = prps.tile([128, 2 * NCLS], f32, name="ps")
                    nc.tensor.matmul(ps[:, 0:NCLS], acc_sum[:, c0:c0 + 128],
                                     wlmeanT_t[:, :], start=True, stop=True)
                    nc.tensor.matmul(ps[:, NCLS:2 * NCLS],
                                     acc_max[:, c0:c0 + 128],
                                     wlmaxT_t[:, :], start=True, stop=False)
                    nc.tensor.matmul(ps[:, NCLS:2 * NCLS], xT_t[:, :],
                                     wrcT_t[:, :], start=False, stop=True)

                    z1 = proj.tile([128, NCLS], f32, name="z1")
                    nc.vector.scalar_tensor_tensor(
                        out=z1[:, :], in0=ps[:, 0:NCLS],
                        scalar=invd_t[:, pc:pc + 1],
                        in1=fix_t[:, pc, :],
                        op0=mybir.AluOpType.mult,
                        op1=mybir.AluOpType.add,
                    )
                    nc.vector.tensor_tensor(zbuf[:, pc, :], z1[:, :],
                                            ps[:, NCLS:2 * NCLS],
                                            mybir.AluOpType.add)

                def emit_softmax(b0, b1):
                    # batched log_softmax over proj chunks [b0, b1): a few
                    # big instructions instead of per-chunk chains, so the
                    # ACT Exp/Ln tables load once per batch, not per chunk
                    nb = b1 - b0
                    nc.vector.tensor_reduce(
                        out=m_t[:, b0:b1], in_=zbuf[:, b0:b1, :],
                        axis=mybir.AxisListType.X, op=mybir.AluOpType.max,
                    )
                    nc.vector.tensor_tensor(
                        zbuf[:, b0:b1, :], zbuf[:, b0:b1, :],
                        m_t[:, b0:b1].broadcast_to([128, nb, NCLS]),
                        mybir.AluOpType.subtract,
                    )
                    nc.scalar.activation(
                        ebuf[:, b0:b1, :], zbuf[:, b0:b1, :],
                        mybir.ActivationFunctionType.Exp,
                    )
                    nc.vector.tensor_reduce(
                        out=se_t[:, b0:b1], in_=ebuf[:, b0:b1, :],
                        axis=mybir.AxisListType.X, op=mybir.AluOpType.add,
                    )
                    nc.scalar.activation(
                        ls_t[:, b0:b1], se_t[:, b0:b1],
                        mybir.ActivationFunctionType.Ln,
                    )
                    nc.vector.tensor_tensor(
                        ebuf[:, b0:b1, :], zbuf[:, b0:b1, :],
                        ls_t[:, b0:b1].broadcast_to([128, nb, NCLS]),
                        mybir.AluOpType.subtract,
                    )
                    nc.sync.dma_start(
                        out=o_out[:, :].rearrange(
                            "(pc p) c -> p pc c", p=128)[:, b0:b1, :],
                        in_=ebuf[:, b0:b1, :],
                    )

                SMB = 12  # softmax batch; 98 % 12 = 2 keeps the final
                # (tail-exposed) batch small
                done = 0
                for ci in range(nchunks):
                    nck = min(IPC, ncols - ci * IPC)
                    g = gpool.tile([128, IPC, D], f32, name="g")
                    for k in range(nck):
                        col = ci * IPC + k
                        nc.gpsimd.indirect_dma_start(
                            out=g[:, k, :],
                            out_offset=None,
                            in_=xg_in[:, :],
                            in_offset=bass.IndirectOffsetOnAxis(
                                ap=idx_t[:, col:col + 1], axis=0
                            ),
                        )
                    pt = ppool.tile([128, CHUNK], f32, name="pt")
                    for b in range(nck):
                        nc.tensor.transpose(
                            pt[:, b * 128:(b + 1) * 128], g[:, b, :], ident_t
                        )
                    for (off, col0, nb, dd, comb) in pieces[ci]:
                        seg = pt[:, off:off + nb * dd].rearrange(
                            "p (nb d) -> p nb d", d=dd
                        )
                        if not comb:
                            nc.vector.tensor_reduce(
                                out=acc_max[:, col0:col0 + nb], in_=seg,
                                axis=mybir.AxisListType.X,
                                op=mybir.AluOpType.max,
                            )
                            nc.vector.tensor_reduce(
                                out=acc_sum[:, col0:col0 + nb], in_=seg,
                                axis=mybir.AxisListType.X,
                                op=mybir.AluOpType.add,
                            )
                        else:
                            # tail fragment of a boundary-split node:
                            # reduce into a temp, then combine into acc
                            frag = proj.tile([128, 2], f32, name="frag")
                            nc.vector.tensor_reduce(
                                out=frag[:, 0:1], in_=seg,
                                axis=mybir.AxisListType.X,
                                op=mybir.AluOpType.max,
                            )
                            nc.vector.tensor_reduce(
                                out=frag[:, 1:2], in_=seg,
                                axis=mybir.AxisListType.X,
                                op=mybir.AluOpType.add,
                            )
                            nc.vector.tensor_tensor(
                                acc_max[:, col0:col0 + 1],
                                acc_max[:, col0:col0 + 1],
                                frag[:, 0:1], mybir.AluOpType.max,
                            )
                            nc.vector.tensor_tensor(
                                acc_sum[:, col0:col0 + 1],
                                acc_sum[:, col0:col0 + 1],
                                frag[:, 1:2], mybir.AluOpType.add,
                            )
                    for pc in proj_after[ci]:
                        emit_proj(pc)
                        done += 1
                        if done % SMB == 0:
                            emit_softmax(done - SMB, done)
                if done % SMB:
                    emit_softmax(done - done % SMB, done)
    return nc


def kernel(**inputs):
    global last_exec_time_ns
    x = np.asarray(inputs["x"], dtype=np.float32)
    ei = np.asarray(inputs["edge_index"]).astype(np.int64)
    Wl_max = np.asarray(inputs["Wl_max"], dtype=np.float32)
    Wr_max = np.asarray(inputs["Wr_max"], dtype=np.float32)
    b_max = np.asarray(inputs["b_max"], dtype=np.float32)
    Wl_mean = np.asarray(inputs["Wl_mean"], dtype=np.float32)
    Wr_mean = np.asarray(inputs["Wr_mean"], dtype=np.float32)
    b_mean = np.asarray(inputs["b_mean"], dtype=np.float32)

    src, dst = ei[0], ei[1]
    (node_core, node_pos, node_of, degs, T, ncols, pieces, proj_b,
     nss) = _plan(dst)
    total_slots = ncols * 128

    xg = np.zeros((N_NODES + 1, D), np.float32)
    xg[1:] = x + SHIFT

    rs = SHIFT * (Wl_max.sum(axis=1) + Wl_mean.sum(axis=1))  # [40]
    bias_eff = b_max + b_mean - rs
    wlmaxT = np.ascontiguousarray(Wl_max.T)
    wlmeanT = np.ascontiguousarray(Wl_mean.T)
    wrcT = np.ascontiguousarray((Wr_max + Wr_mean).T)

    core_of_dst = node_core[dst]
    pos_of_dst = node_pos[dst]
    in_maps = []
    for c in range(NCORES):
        msk = core_of_dst == c
        idx = _core_idx(src[msk], pos_of_dst[msk], degs[c],
                        nss, total_slots)
        idx_t = np.ascontiguousarray(
            idx.reshape(ncols, 128).T).astype(np.int32)

        ids = node_of[c]
        real = ids >= 0
        xo = np.zeros((NPAD, D), np.float32)
        xo[real] = x[ids[real]]
        xT = np.ascontiguousarray(xo.T)

        invd = (1.0 / np.maximum(degs[c], 1)).astype(np.float32)
        invd_t = np.ascontiguousarray(invd.reshape(NPROJ, 128).T)

        fix = np.tile(bias_eff, (NPAD, 1)).astype(np.float32)
        fix[degs[c] == 0] += rs
        fix_t = np.ascontiguousarray(
            fix.reshape(NPROJ, 128, NCLS).transpose(1, 0, 2))

        in_maps.append({
            "xg": xg, "idx": idx_t, "xT": xT, "invd": invd_t, "fix": fix_t,
            "wlmaxT": wlmaxT, "wlmeanT": wlmeanT, "wrcT": wrcT,
        })

    nc = _build_program(ncols, pieces, proj_b)
    nc.compile()

    from concourse.bass_utils import run_bass_kernel_spmd
    res = run_bass_kernel_spmd(nc, in_maps, list(range(NCORES)))
    if os.environ.get("GNN_TRACE", "0") == "1":
        # separate single-core traced run: tracing the 8-core run crashes
        # the exec unit; core 0's time is representative (identical program)
        tr = run_bass_kernel_spmd(nc, in_maps[:1], [0], trace=True)
        last_exec_time_ns = tr.exec_time_ns

    out = np.zeros((N_NODES, NCLS), np.float32)
    for c in range(NCORES):
        o = np.asarray(res.results[c]["out"])
        ids = node_of[c]
        real = ids >= 0
        out[ids[real]] = o[real]
    return out

